# revision 19
# baseline (speedup 1.0000x reference)
"""Chamfer + edge + normal-cosine combined loss on 8 Trainium2 cores.

v2: candidate-pruned scan. Host kd-sorts both point sets per batch, computes
bbox lower bounds LB(t-tile, p-group) and per-point upper bounds on nearest
distances, and keeps only (tile, group) pairs that can contain a row-min
(LB <= UB_tile) or a column-min (LB <= UB_group) -- provably covering every
exact row/column argmin (~19% of all pairs survive). Kept fine groups (16 pts)
are packed into dense 512-column chunks; the device is a pure streaming
scanner: per step, four K=24 row-tiled bf16 matmuls (tile_position row groups
32j, bf16 3-way-split factors reproduce fp32-accurate dot products) fill one
[128, 2048] PSUM buffer, ACT/DVE alternate casting PSUM->bf16 staging, and the
chunk values ship to DRAM on two DMA queues. Host finishes: row/col maxes over
the shipped bf16 chunks select argmins, final distances are recomputed exactly,
and the tiny edge/normal-cosine terms run in numpy as before.
"""

from contextlib import ExitStack

import ml_dtypes
import numpy as np

B = 4
N = 8192
NCORES = 8
TIL = 128            # t rows per tile (partition dim)
GRP = 8              # fine p-group size for pruning
CHW = 512            # chunk width in columns
GPC = CHW // GRP     # 32 fine groups per chunk
NT = N // TIL        # 64 t-tiles per batch
NGR = N // GRP       # 512 fine groups per batch
KS = 24              # bf16 split rows (3-way split, as baseline)
NNEAR = 5            # groups sampled for upper bounds

_LAST_RESULTS = {}


# ---------------------------------------------------------------- host: split
def _split3(x):
    h = x.astype(ml_dtypes.bfloat16)
    r1 = x - h.astype(np.float32)
    m = r1.astype(ml_dtypes.bfloat16)
    r2 = r1 - m.astype(np.float32)
    l = r2.astype(ml_dtypes.bfloat16)
    return h, m, l


def _build_split_rows(L, R):
    """L [5, X], R [5, Y] fp32 term rows -> bf16 [24, X], [24, Y].

    M = sum_k L[k] (outer) R[k] = 2<g,p> - |g|^2 - |p|^2 = -P."""
    outL, outR = [], []
    for c in range(3):
        Lh, Lm, Ll = _split3(L[c])
        Rh, Rm, Rl = _split3(R[c])
        for a, b in ((Lh, Rh), (Lh, Rm), (Lm, Rh), (Lh, Rl), (Ll, Rh), (Lm, Rm)):
            outL.append(a)
            outR.append(b)
    Xh, Xm, Xl = _split3(L[3])
    negone = R[3].astype(ml_dtypes.bfloat16)
    for a in (Xh, Xm, Xl):
        outL.append(a)
        outR.append(negone)
    Yh, Ym, Yl = _split3(R[4])
    one = L[4].astype(ml_dtypes.bfloat16)
    for b in (Yh, Ym, Yl):
        outL.append(one)
        outR.append(b)
    return np.ascontiguousarray(np.stack(outL)), np.ascontiguousarray(np.stack(outR))


# -------------------------------------------------------------- host: pruning
def _kd_order(pts, leaf):
    """Balanced kd-tree order: median split on widest axis down to `leaf`."""
    out = []
    stack = [np.arange(len(pts))]
    while stack:
        ids = stack.pop()
        if len(ids) <= leaf:
            out.append(ids)
            continue
        p = pts[ids]
        ax = int((p.max(0) - p.min(0)).argmax())
        k = len(ids) // 2
        o = np.argpartition(p[:, ax], k)
        stack.append(ids[o[k:]])
        stack.append(ids[o[:k]])
    # stack order: first-pushed-last; rebuild in left-to-right order
    return np.concatenate(out)


def _point_ubs(A, Btiles, nnear):
    """For each point in A [n,3]: an achievable nearest-distance^2 upper bound,
    the min over all points of the `nnear` nearest B-tiles (by center)."""
    bc = Btiles.mean(1)
    d = ((A[:, None, :] - bc[None, :, :]) ** 2).sum(-1)
    near = np.argpartition(d, nnear, axis=1)[:, :nnear]
    ub = np.full(len(A), np.inf)
    for j in range(nnear):
        sel = near[:, j]
        for g in np.unique(sel):
            m = sel == g
            dd = ((A[m][:, None, :] - Btiles[g][None, :, :]) ** 2).sum(-1).min(1)
            ub[m] = np.minimum(ub[m], dd)
    return ub


def _prep_batch(preds_b, gts_b):
    """Returns sorted perms, per-tile candidate chunk lists and split tables."""
    po = _kd_order(preds_b, GRP)
    go = _kd_order(gts_b, TIL)
    Ps = preds_b[po].astype(np.float64)
    Gs = gts_b[go].astype(np.float64)

    Pt = Ps.reshape(NGR, GRP, 3)
    Gt = Gs.reshape(NT, TIL, 3)
    plo, phi = Pt.min(1), Pt.max(1)
    glo, ghi = Gt.min(1), Gt.max(1)
    d1 = np.maximum(0.0, plo[None, :, :] - ghi[:, None, :])
    d2 = np.maximum(0.0, glo[:, None, :] - phi[None, :, :])
    LB = (np.maximum(d1, d2) ** 2).sum(-1)           # [NT, NGR]

    ub_t = _point_ubs(Gs, Pt, NNEAR)
    UB_T = ub_t.reshape(NT, TIL).max(1)              # [NT]
    ub_p = _point_ubs(Ps, Gt, NNEAR)
    UB_G = ub_p.reshape(NGR, GRP).max(1)             # [NGR]

    keep = LB <= np.maximum(UB_T[:, None], UB_G[None, :]) * (1.0 + 1e-6) + 1e-12

    # chunk lists: per tile, its fine groups packed into CHW-wide chunks
    chunks = []                                      # (tile, group_ids[GPC])
    for T in range(NT):
        gl = np.nonzero(keep[T])[0]
        padded = ((len(gl) + GPC - 1) // GPC) * GPC
        gl = np.resize(gl, padded)      # cycles values to pad
        for c in range(len(gl) // GPC):
            chunks.append((T, gl[c * GPC:(c + 1) * GPC]))

    # split tables over sorted points
    xsq = (Gs * Gs).sum(-1).astype(np.float32)
    ysq = (Ps * Ps).sum(-1).astype(np.float32)
    L = np.empty((5, N), np.float32)
    L[0:3] = (2.0 * Gs.T).astype(np.float32)
    L[3] = xsq
    L[4] = 1.0
    R = np.empty((5, N), np.float32)
    R[0:3] = Ps.T.astype(np.float32)
    R[3] = -1.0
    R[4] = -ysq
    sL, sR = _build_split_rows(L, R)                 # [24, N] bf16 each
    return dict(po=po, go=go, chunks=chunks, sL=sL, sR=sR)


def _prep(preds, gts):
    metas = [_prep_batch(preds[b], gts[b]) for b in range(B)]
    # distribute chunks to cores: core = 2*b + (tile >= 32)
    raw = []
    for b in range(B):
        for h in range(2):
            lo, hi = h * 32, (h + 1) * 32
            raw.append([ch for ch in metas[b]['chunks'] if lo <= ch[0] < hi])
    nreal = [len(cc) for cc in raw]
    steps = max((n + 3) // 4 for n in nreal)
    steps = ((steps + 3) // 4) * 4   # multiple of 4 for DMA batching
    in_maps = []
    core_data = []
    for c in range(NCORES):
        b = c // 2
        sL, sR = metas[b]['sL'], metas[b]['sR']
        cc = list(raw[c])
        while len(cc) < steps * 4:
            cc.append(cc[-1])
        in0 = np.zeros((steps // 4, 128, 2560), ml_dtypes.bfloat16)
        colmap = np.empty((steps * 4, CHW), np.int32)
        tileof = np.empty(steps * 4, np.int32)
        for i, (T, gl) in enumerate(cc):
            cols = (gl[:, None] * GRP + np.arange(GRP)[None, :]).ravel()
            s, j = divmod(i, 4)
            s4, p = divmod(s, 4)
            o = p * 640
            in0[s4, 32 * j:32 * j + KS, o:o + 128] = sL[:, T * TIL:(T + 1) * TIL]
            in0[s4, 32 * j:32 * j + KS, o + 128:o + 640] = sR[:, cols]
            colmap[i] = cols
            tileof[i] = T
        in_maps.append({"in0": in0})
        core_data.append((cc, colmap, tileof))
    return metas, core_data, nreal, steps, in_maps


# ------------------------------------------------------------------- device
def _build_nc(steps):
    import concourse.mybir as mybir
    import concourse.tile as tile
    from concourse import bacc

    f32 = mybir.dt.float32
    bf16 = mybir.dt.bfloat16
    nc = bacc.Bacc("TRN2", target_bir_lowering=False, debug=False)

    # inputs batched 4 steps per DMA; outputs: each caster fills a [128, 4096]
    # stage over 4 steps (8 KB DMA lines), shipped on its own queue
    in0_d = nc.dram_tensor("in0", [steps // 4, 128, 2560], bf16, kind="ExternalInput")
    outa_d = nc.dram_tensor("outa", [steps // 4, 128, 4096], bf16, kind="ExternalOutput")
    outb_d = nc.dram_tensor("outb", [steps // 4, 128, 4096], bf16, kind="ExternalOutput")

    with tile.TileContext(nc) as tc, ExitStack() as ctx:
        io_pool = ctx.enter_context(tc.tile_pool(name="io", bufs=3))
        psum_pool = ctx.enter_context(tc.tile_pool(name="psum", bufs=2, space="PSUM"))
        stage_pool = ctx.enter_context(tc.tile_pool(name="stage", bufs=2))

        for s4 in range(steps // 4):
            t_in = io_pool.tile([128, 2560], bf16)
            nc.scalar.dma_start(t_in[:], in0_d[s4, :, :])
            sta = stage_pool.tile([128, 4096], bf16, tag="sta")
            stb = stage_pool.tile([128, 4096], bf16, tag="stb")
            for p in range(4):
                ps = psum_pool.tile([128, 2048], f32, tag="ps")
                for j in range(4):
                    nc.tensor.matmul(
                        ps[:, j * 512:(j + 1) * 512],
                        t_in[32 * j:32 * j + KS, p * 640:p * 640 + 128],
                        t_in[32 * j:32 * j + KS, p * 640 + 128:p * 640 + 640],
                        start=True,
                        stop=True,
                        tile_position=(32 * j, 0),
                    )
                nc.scalar.copy(sta[:, p * 1024:(p + 1) * 1024], ps[:, 0:1024])
                nc.vector.tensor_copy(stb[:, p * 1024:(p + 1) * 1024], ps[:, 1024:2048])
            nc.gpsimd.dma_start(outa_d[s4, :, :], sta[:])
            nc.sync.dma_start(outb_d[s4, :, :], stb[:])

    nc.compile()
    return nc


# ------------------------------------------------------------------ host: post
def _postprocess(preds, gts, normals, edges, results, metas, core_chunks, nreal):
    preds64 = preds.astype(np.float64)
    gts64 = gts.astype(np.float64)

    mins1 = np.empty((B, N), np.float64)
    mins2 = np.empty((B, N), np.float64)
    nearest_idx = np.empty((B, N), np.int64)

    for b in range(B):
        po, go = metas[b]['po'], metas[b]['go']
        # gather both cores' chunk values for this batch
        vals_all, cols_all, tile_all = [], [], []
        for h in range(2):
            c = 2 * b + h
            cc, colmap, tileof = core_chunks[c]
            # outa holds chunks j=0,1 of each step; outb j=2,3
            va = np.asarray(results[c]["outa"], ml_dtypes.bfloat16).astype(np.float32)
            vb = np.asarray(results[c]["outb"], ml_dtypes.bfloat16).astype(np.float32)
            # [s4, 128, 4(p), 2(jj), 512] -> [steps*2, 128, 512] in (s, jj) order
            va = va.reshape(-1, 128, 4, 2, CHW).transpose(0, 2, 3, 1, 4).reshape(-1, 2, 128, CHW)
            vb = vb.reshape(-1, 128, 4, 2, CHW).transpose(0, 2, 3, 1, 4).reshape(-1, 2, 128, CHW)
            v = np.concatenate([va, vb], axis=1).reshape(-1, 128, CHW)
            v = v[:nreal[c]]                             # [nch, 128, 512]
            vals_all.append(v)
            cols_all.append(colmap[:nreal[c]])
            tile_all.append(tileof[:nreal[c]])
        vals = np.concatenate(vals_all)                  # [M, 128, 512]
        cols = np.concatenate(cols_all)                  # [M, 512] sorted-p idx
        tils = np.concatenate(tile_all)                  # [M]

        # ---- row path: per tile, max over its chunks' columns
        order = np.argsort(tils, kind='stable')
        vals_o, cols_o, tils_o = vals[order], cols[order], tils[order]
        bounds = np.searchsorted(tils_o, np.arange(NT + 1))
        for T in range(NT):
            i0, i1 = bounds[T], bounds[T + 1]
            v = vals_o[i0:i1]                            # [m, 128, 512]
            flat = v.transpose(1, 0, 2).reshape(TIL, -1)
            am = flat.argmax(1)                          # [128]
            ci, cj = divmod(am, CHW)
            srt_p = cols_o[i0:i1][ci, cj]                # sorted-p index
            t_orig = go[T * TIL + np.arange(TIL)]
            p_orig = po[srt_p]
            d = ((gts64[b, t_orig] - preds64[b, p_orig]) ** 2).sum(-1)
            mins2[b, t_orig] = d
            nearest_idx[b, t_orig] = p_orig

        # ---- col path: per sorted-p column, max over all (chunk, t)
        cmax = vals.max(1)                               # [M, 512]
        cargt = vals.argmax(1)                           # [M, 512] best t-row
        flat_cols = cols.ravel()
        flat_vals = cmax.ravel()
        # global t index of each entry's best row
        trow = (tils[:, None] * TIL + cargt).ravel()     # sorted-t index
        o2 = np.lexsort((-flat_vals, flat_cols))
        fc, first = np.unique(flat_cols[o2], return_index=True)
        assert len(fc) == N, "column coverage hole"
        sel = o2[first]
        srt_t = trow[sel]
        p_orig = po[fc]
        t_orig = go[srt_t]
        d = ((gts64[b, t_orig] - preds64[b, p_orig]) ** 2).sum(-1)
        mins1[b, p_orig] = d

    loss_1 = mins1.mean()
    loss_2 = mins2.mean()
    chamfer = loss_1 + loss_2

    e0 = edges[:, 0]
    e1 = edges[:, 1]
    edge_vectors = preds[:, e0, :] - preds[:, e1, :]
    edge_loss = (edge_vectors * edge_vectors).sum(axis=2).astype(np.float64).mean()

    normals_nearest = np.take_along_axis(normals, nearest_idx[:, :, None], axis=1)
    normals_edge = normals_nearest[:, e0, :]

    def l2n_dim1(v):
        n = np.sqrt((v * v).sum(axis=1, keepdims=True))
        return v / np.maximum(n, 1e-12)

    nn = l2n_dim1(normals_edge)
    nv = l2n_dim1(edge_vectors)
    cosines = np.abs((nn * nv).sum(axis=2))
    normal_cosine_loss = cosines.astype(np.float64).mean()

    return np.float32(
        30000.0 * chamfer + 240.0 * edge_loss + 200000.0 * normal_cosine_loss
    )


def kernel(preds, gts, normals, edges, _trace=False):
    from concourse.bass_utils import run_bass_kernel_spmd

    preds = np.asarray(preds, np.float32)
    gts = np.asarray(gts, np.float32)
    normals = np.asarray(normals, np.float32)
    edges = np.asarray(edges)

    metas, core_data, nreal, steps, in_maps = _prep(preds, gts)
    nc = _build_nc(steps)
    br = run_bass_kernel_spmd(nc, in_maps, list(range(NCORES)), trace=_trace)
    _LAST_RESULTS["bass_results"] = br
    return _postprocess(preds, gts, normals, edges, br.results,
                        metas, core_data, nreal)


# revision 23
# speedup vs baseline: 1.2169x; 1.2169x over previous
"""Chamfer + edge + normal-cosine combined loss on 8 Trainium2 cores.

v2: candidate-pruned scan. Host kd-sorts both point sets per batch, computes
bbox lower bounds LB(t-tile, p-group) and per-point upper bounds on nearest
distances, and keeps only (tile, group) pairs that can contain a row-min
(LB <= UB_tile) or a column-min (LB <= UB_group) -- provably covering every
exact row/column argmin (~19% of all pairs survive). Kept fine groups (16 pts)
are packed into dense 512-column chunks; the device is a pure streaming
scanner: per step, four K=24 row-tiled bf16 matmuls (tile_position row groups
32j, bf16 3-way-split factors reproduce fp32-accurate dot products) fill one
[128, 2048] PSUM buffer, ACT/DVE alternate casting PSUM->bf16 staging, and the
chunk values ship to DRAM on two DMA queues. Host finishes: row/col maxes over
the shipped bf16 chunks select argmins, final distances are recomputed exactly,
and the tiny edge/normal-cosine terms run in numpy as before.
"""

from contextlib import ExitStack

import ml_dtypes
import numpy as np

B = 4
N = 8192
NCORES = 8
TIL = 64             # t rows per tile (two tiles stack in 128 partitions)
GRP = 8              # fine p-group size for pruning
CHW = 512            # chunk width in columns
GPC = CHW // GRP     # 32 fine groups per chunk
NT = N // TIL        # 64 t-tiles per batch
NGR = N // GRP       # 512 fine groups per batch
KS = 24              # bf16 split rows (3-way split, as baseline)
NNEAR = 5            # groups sampled for upper bounds

_LAST_RESULTS = {}


# ---------------------------------------------------------------- host: split
def _split3(x):
    h = x.astype(ml_dtypes.bfloat16)
    r1 = x - h.astype(np.float32)
    m = r1.astype(ml_dtypes.bfloat16)
    r2 = r1 - m.astype(np.float32)
    l = r2.astype(ml_dtypes.bfloat16)
    return h, m, l


def _build_split_rows(L, R):
    """L [5, X], R [5, Y] fp32 term rows -> bf16 [24, X], [24, Y].

    M = sum_k L[k] (outer) R[k] = 2<g,p> - |g|^2 - |p|^2 = -P."""
    outL, outR = [], []
    for c in range(3):
        Lh, Lm, Ll = _split3(L[c])
        Rh, Rm, Rl = _split3(R[c])
        for a, b in ((Lh, Rh), (Lh, Rm), (Lm, Rh), (Lh, Rl), (Ll, Rh), (Lm, Rm)):
            outL.append(a)
            outR.append(b)
    Xh, Xm, Xl = _split3(L[3])
    negone = R[3].astype(ml_dtypes.bfloat16)
    for a in (Xh, Xm, Xl):
        outL.append(a)
        outR.append(negone)
    Yh, Ym, Yl = _split3(R[4])
    one = L[4].astype(ml_dtypes.bfloat16)
    for b in (Yh, Ym, Yl):
        outL.append(one)
        outR.append(b)
    return np.ascontiguousarray(np.stack(outL)), np.ascontiguousarray(np.stack(outR))


# -------------------------------------------------------------- host: pruning
def _kd_order(pts, leaf):
    """Balanced kd-tree order: median split on widest axis down to `leaf`."""
    out = []
    stack = [np.arange(len(pts))]
    while stack:
        ids = stack.pop()
        if len(ids) <= leaf:
            out.append(ids)
            continue
        p = pts[ids]
        ax = int((p.max(0) - p.min(0)).argmax())
        k = len(ids) // 2
        o = np.argpartition(p[:, ax], k)
        stack.append(ids[o[k:]])
        stack.append(ids[o[:k]])
    # stack order: first-pushed-last; rebuild in left-to-right order
    return np.concatenate(out)


def _point_ubs(A, Btiles, nnear):
    """For each point in A [n,3]: an achievable nearest-distance^2 upper bound,
    the min over all points of the `nnear` nearest B-tiles (by center)."""
    bc = Btiles.mean(1)
    d = ((A[:, None, :] - bc[None, :, :]) ** 2).sum(-1)
    near = np.argpartition(d, nnear, axis=1)[:, :nnear]
    ub = np.full(len(A), np.inf)
    for j in range(nnear):
        sel = near[:, j]
        for g in np.unique(sel):
            m = sel == g
            dd = ((A[m][:, None, :] - Btiles[g][None, :, :]) ** 2).sum(-1).min(1)
            ub[m] = np.minimum(ub[m], dd)
    return ub


def _prep_batch(preds_b, gts_b):
    """Returns sorted perms, per-tile candidate chunk lists and split tables."""
    po = _kd_order(preds_b, GRP)
    go = _kd_order(gts_b, TIL)
    Ps = preds_b[po].astype(np.float64)
    Gs = gts_b[go].astype(np.float64)

    Pt = Ps.reshape(NGR, GRP, 3)
    Gt = Gs.reshape(NT, TIL, 3)
    plo, phi = Pt.min(1), Pt.max(1)
    glo, ghi = Gt.min(1), Gt.max(1)
    d1 = np.maximum(0.0, plo[None, :, :] - ghi[:, None, :])
    d2 = np.maximum(0.0, glo[:, None, :] - phi[None, :, :])
    LB = (np.maximum(d1, d2) ** 2).sum(-1)           # [NT, NGR]

    ub_t = _point_ubs(Gs, Pt, NNEAR)
    UB_T = ub_t.reshape(NT, TIL).max(1)              # [NT]
    ub_p = _point_ubs(Ps, Gt, NNEAR)
    UB_G = ub_p.reshape(NGR, GRP).max(1)             # [NGR]

    keep = LB <= np.maximum(UB_T[:, None], UB_G[None, :]) * (1.0 + 1e-6) + 1e-12

    # chunk lists: per tile, its fine groups packed into CHW-wide chunks
    chunks = []                                      # (tile, group_ids[GPC])
    for T in range(NT):
        gl = np.nonzero(keep[T])[0]
        padded = ((len(gl) + GPC - 1) // GPC) * GPC
        gl = np.resize(gl, padded)      # cycles values to pad
        for c in range(len(gl) // GPC):
            chunks.append((T, gl[c * GPC:(c + 1) * GPC]))

    # split tables over sorted points
    xsq = (Gs * Gs).sum(-1).astype(np.float32)
    ysq = (Ps * Ps).sum(-1).astype(np.float32)
    L = np.empty((5, N), np.float32)
    L[0:3] = (2.0 * Gs.T).astype(np.float32)
    L[3] = xsq
    L[4] = 1.0
    R = np.empty((5, N), np.float32)
    R[0:3] = Ps.T.astype(np.float32)
    R[3] = -1.0
    R[4] = -ysq
    sL, sR = _build_split_rows(L, R)                 # [24, N] bf16 each
    return dict(po=po, go=go, chunks=chunks, sL=sL, sR=sR)


def _prep(preds, gts):
    metas = [_prep_batch(preds[b], gts[b]) for b in range(B)]
    # distribute chunks to cores: core = 2*b + (tile >= NT//2)
    raw = []
    for b in range(B):
        for h in range(2):
            lo, hi = h * (NT // 2), (h + 1) * (NT // 2)
            raw.append([ch for ch in metas[b]['chunks'] if lo <= ch[0] < hi])
    nreal = [len(cc) for cc in raw]
    # 8 chunks per step (4 row-groups x 2 col-groups); 2 steps per in-DMA
    steps = max((n + 7) // 8 for n in nreal)
    steps += steps % 2
    in_maps = []
    core_data = []
    for c in range(NCORES):
        b = c // 2
        sL, sR = metas[b]['sL'], metas[b]['sR']
        cc = list(raw[c])
        while len(cc) < steps * 8:
            cc.append(cc[-1])
        # per step 1152 cols: slot (k, j): j-block partitions 32j+[0,24),
        # cols phase*1152 + k*576 + [0:64 weights | 64:576 rhs]
        in0 = np.zeros((steps // 2, 128, 2304), ml_dtypes.bfloat16)
        colmap = np.empty((steps * 8, CHW), np.int32)
        tileof = np.empty(steps * 8, np.int32)
        for i, (T, gl) in enumerate(cc):
            cols = (gl[:, None] * GRP + np.arange(GRP)[None, :]).ravel()
            s, slot = divmod(i, 8)
            k, j = divmod(slot, 4)
            s2, ph = divmod(s, 2)
            o = ph * 1152 + k * 576
            in0[s2, 32 * j:32 * j + KS, o:o + 64] = sL[:, T * TIL:(T + 1) * TIL]
            in0[s2, 32 * j:32 * j + KS, o + 64:o + 576] = sR[:, cols]
            colmap[i] = cols
            tileof[i] = T
        in_maps.append({"in0": in0})
        core_data.append((cc, colmap, tileof))
    return metas, core_data, nreal, steps, in_maps


# ------------------------------------------------------------------- device
def _build_nc(steps):
    import concourse.mybir as mybir
    import concourse.tile as tile
    from concourse import bacc

    f32 = mybir.dt.float32
    bf16 = mybir.dt.bfloat16
    nc = bacc.Bacc("TRN2", target_bir_lowering=False, debug=False)

    # inputs batched 2 steps per DMA; per step 8 chunks of [64, 512]:
    # row-groups 32j (K=24) x col-groups 64k. ACT casts psA (chunks j=0,1),
    # DVE casts psB (j=2,3) -- separate PSUM tiles so the casts run parallel.
    in0_d = nc.dram_tensor("in0", [steps // 2, 128, 2304], bf16, kind="ExternalInput")
    outa_d = nc.dram_tensor("outa", [steps // 2, 128, 2048], bf16, kind="ExternalOutput")
    outb_d = nc.dram_tensor("outb", [steps // 2, 128, 2048], bf16, kind="ExternalOutput")

    with tile.TileContext(nc) as tc, ExitStack() as ctx:
        io_pool = ctx.enter_context(tc.tile_pool(name="io", bufs=3))
        psum_pool = ctx.enter_context(tc.tile_pool(name="psum", bufs=2, space="PSUM"))
        stage_pool = ctx.enter_context(tc.tile_pool(name="stage", bufs=2))

        for s2 in range(steps // 2):
            t_in = io_pool.tile([128, 2304], bf16)
            nc.scalar.dma_start(t_in[:], in0_d[s2, :, :])
            sta = stage_pool.tile([128, 2048], bf16, tag="sta")
            stb = stage_pool.tile([128, 2048], bf16, tag="stb")
            for ph in range(2):
                psa = psum_pool.tile([128, 1024], f32, tag="psa")
                psb = psum_pool.tile([128, 1024], f32, tag="psb")
                for k in range(2):
                    for j in range(4):
                        o = ph * 1152 + k * 576
                        ps = psa if j < 2 else psb
                        nc.tensor.matmul(
                            ps[64 * k:64 * k + 64, (j % 2) * 512:(j % 2 + 1) * 512],
                            t_in[32 * j:32 * j + KS, o:o + 64],
                            t_in[32 * j:32 * j + KS, o + 64:o + 576],
                            start=True,
                            stop=True,
                            tile_position=(32 * j, 64 * k),
                        )
                nc.scalar.copy(sta[:, ph * 1024:(ph + 1) * 1024], psa[:])
                nc.vector.tensor_copy(stb[:, ph * 1024:(ph + 1) * 1024], psb[:])
            nc.gpsimd.dma_start(outa_d[s2, :, :], sta[:])
            nc.sync.dma_start(outb_d[s2, :, :], stb[:])

    nc.compile()
    return nc


# ------------------------------------------------------------------ host: post
def _postprocess(preds, gts, normals, edges, results, metas, core_chunks, nreal):
    preds64 = preds.astype(np.float64)
    gts64 = gts.astype(np.float64)

    mins1 = np.empty((B, N), np.float64)
    mins2 = np.empty((B, N), np.float64)
    nearest_idx = np.empty((B, N), np.int64)

    for b in range(B):
        po, go = metas[b]['po'], metas[b]['go']
        # gather both cores' chunk values for this batch
        vals_all, cols_all, tile_all = [], [], []
        for h in range(2):
            c = 2 * b + h
            cc, colmap, tileof = core_chunks[c]
            # outa holds chunks j=0,1 of each step (k-halves in partitions);
            # outb j=2,3. chunk i = s*8 + k*4 + j.
            va = np.asarray(results[c]["outa"], ml_dtypes.bfloat16).astype(np.float32)
            vb = np.asarray(results[c]["outb"], ml_dtypes.bfloat16).astype(np.float32)
            # [s2, 2(k), 64, 2(ph), 2(j2), 512] -> [(s,k), j2, 64, 512]
            va = va.reshape(-1, 2, TIL, 2, 2, CHW).transpose(0, 3, 1, 4, 2, 5)
            vb = vb.reshape(-1, 2, TIL, 2, 2, CHW).transpose(0, 3, 1, 4, 2, 5)
            va = va.reshape(-1, 2, 2, TIL, CHW)          # [s, k, j2, 64, 512]
            vb = vb.reshape(-1, 2, 2, TIL, CHW)
            v = np.concatenate([va, vb], axis=2).reshape(-1, TIL, CHW)
            v = v[:nreal[c]]                             # [nch, 64, 512]
            vals_all.append(v)
            cols_all.append(colmap[:nreal[c]])
            tile_all.append(tileof[:nreal[c]])
        vals = np.concatenate(vals_all)                  # [M, 128, 512]
        cols = np.concatenate(cols_all)                  # [M, 512] sorted-p idx
        tils = np.concatenate(tile_all)                  # [M]

        # ---- row path: per tile, max over its chunks' columns
        order = np.argsort(tils, kind='stable')
        vals_o, cols_o, tils_o = vals[order], cols[order], tils[order]
        bounds = np.searchsorted(tils_o, np.arange(NT + 1))
        for T in range(NT):
            i0, i1 = bounds[T], bounds[T + 1]
            v = vals_o[i0:i1]                            # [m, 128, 512]
            flat = v.transpose(1, 0, 2).reshape(TIL, -1)
            am = flat.argmax(1)                          # [128]
            ci, cj = divmod(am, CHW)
            srt_p = cols_o[i0:i1][ci, cj]                # sorted-p index
            t_orig = go[T * TIL + np.arange(TIL)]
            p_orig = po[srt_p]
            d = ((gts64[b, t_orig] - preds64[b, p_orig]) ** 2).sum(-1)
            mins2[b, t_orig] = d
            nearest_idx[b, t_orig] = p_orig

        # ---- col path: per sorted-p column, max over all (chunk, t)
        cmax = vals.max(1)                               # [M, 512]
        cargt = vals.argmax(1)                           # [M, 512] best t-row
        flat_cols = cols.ravel()
        flat_vals = cmax.ravel()
        # global t index of each entry's best row
        trow = (tils[:, None] * TIL + cargt).ravel()     # sorted-t index
        o2 = np.lexsort((-flat_vals, flat_cols))
        fc, first = np.unique(flat_cols[o2], return_index=True)
        assert len(fc) == N, "column coverage hole"
        sel = o2[first]
        srt_t = trow[sel]
        p_orig = po[fc]
        t_orig = go[srt_t]
        d = ((gts64[b, t_orig] - preds64[b, p_orig]) ** 2).sum(-1)
        mins1[b, p_orig] = d

    loss_1 = mins1.mean()
    loss_2 = mins2.mean()
    chamfer = loss_1 + loss_2

    e0 = edges[:, 0]
    e1 = edges[:, 1]
    edge_vectors = preds[:, e0, :] - preds[:, e1, :]
    edge_loss = (edge_vectors * edge_vectors).sum(axis=2).astype(np.float64).mean()

    normals_nearest = np.take_along_axis(normals, nearest_idx[:, :, None], axis=1)
    normals_edge = normals_nearest[:, e0, :]

    def l2n_dim1(v):
        n = np.sqrt((v * v).sum(axis=1, keepdims=True))
        return v / np.maximum(n, 1e-12)

    nn = l2n_dim1(normals_edge)
    nv = l2n_dim1(edge_vectors)
    cosines = np.abs((nn * nv).sum(axis=2))
    normal_cosine_loss = cosines.astype(np.float64).mean()

    return np.float32(
        30000.0 * chamfer + 240.0 * edge_loss + 200000.0 * normal_cosine_loss
    )


def kernel(preds, gts, normals, edges, _trace=False):
    from concourse.bass_utils import run_bass_kernel_spmd

    preds = np.asarray(preds, np.float32)
    gts = np.asarray(gts, np.float32)
    normals = np.asarray(normals, np.float32)
    edges = np.asarray(edges)

    metas, core_data, nreal, steps, in_maps = _prep(preds, gts)
    nc = _build_nc(steps)
    br = run_bass_kernel_spmd(nc, in_maps, list(range(NCORES)), trace=_trace)
    _LAST_RESULTS["bass_results"] = br
    return _postprocess(preds, gts, normals, edges, br.results,
                        metas, core_data, nreal)


# revision 28
# speedup vs baseline: 1.3799x; 1.1340x over previous
"""Chamfer + edge + normal-cosine combined loss on 8 Trainium2 cores.

v2: candidate-pruned scan. Host kd-sorts both point sets per batch, computes
bbox lower bounds LB(t-tile, p-group) and per-point upper bounds on nearest
distances, and keeps only (tile, group) pairs that can contain a row-min
(LB <= UB_tile) or a column-min (LB <= UB_group) -- provably covering every
exact row/column argmin (~19% of all pairs survive). Kept fine groups (16 pts)
are packed into dense 512-column chunks; the device is a pure streaming
scanner: per step, four K=24 row-tiled bf16 matmuls (tile_position row groups
32j, bf16 3-way-split factors reproduce fp32-accurate dot products) fill one
[128, 2048] PSUM buffer, ACT/DVE alternate casting PSUM->bf16 staging, and the
chunk values ship to DRAM on two DMA queues. Host finishes: row/col maxes over
the shipped bf16 chunks select argmins, final distances are recomputed exactly,
and the tiny edge/normal-cosine terms run in numpy as before.
"""

from contextlib import ExitStack

import ml_dtypes
import numpy as np

B = 4
N = 8192
NCORES = 8
TIL = 64             # t rows per tile (two tiles stack in 128 partitions)
GRP = 8              # fine p-group size for pruning
CHW = 512            # chunk width in columns
GPC = CHW // GRP     # 32 fine groups per chunk
NT = N // TIL        # 64 t-tiles per batch
NGR = N // GRP       # 512 fine groups per batch
KS = 24              # bf16 split rows (3-way split, as baseline)
NNEAR = 5            # groups sampled for upper bounds

_LAST_RESULTS = {}


# ---------------------------------------------------------------- host: split
def _split3(x):
    h = x.astype(ml_dtypes.bfloat16)
    r1 = x - h.astype(np.float32)
    m = r1.astype(ml_dtypes.bfloat16)
    r2 = r1 - m.astype(np.float32)
    l = r2.astype(ml_dtypes.bfloat16)
    return h, m, l


def _build_split_rows(L, R):
    """L [5, X], R [5, Y] fp32 term rows -> bf16 [24, X], [24, Y].

    M = sum_k L[k] (outer) R[k] = 2<g,p> - |g|^2 - |p|^2 = -P."""
    outL, outR = [], []
    for c in range(3):
        Lh, Lm, Ll = _split3(L[c])
        Rh, Rm, Rl = _split3(R[c])
        for a, b in ((Lh, Rh), (Lh, Rm), (Lm, Rh), (Lh, Rl), (Ll, Rh), (Lm, Rm)):
            outL.append(a)
            outR.append(b)
    Xh, Xm, Xl = _split3(L[3])
    negone = R[3].astype(ml_dtypes.bfloat16)
    for a in (Xh, Xm, Xl):
        outL.append(a)
        outR.append(negone)
    Yh, Ym, Yl = _split3(R[4])
    one = L[4].astype(ml_dtypes.bfloat16)
    for b in (Yh, Ym, Yl):
        outL.append(one)
        outR.append(b)
    return np.ascontiguousarray(np.stack(outL)), np.ascontiguousarray(np.stack(outR))


# -------------------------------------------------------------- host: pruning
def _kd_order(pts, leaf):
    """Balanced kd-tree order: median split on widest axis down to `leaf`."""
    out = []
    stack = [np.arange(len(pts))]
    while stack:
        ids = stack.pop()
        if len(ids) <= leaf:
            out.append(ids)
            continue
        p = pts[ids]
        ax = int((p.max(0) - p.min(0)).argmax())
        k = len(ids) // 2
        o = np.argpartition(p[:, ax], k)
        stack.append(ids[o[k:]])
        stack.append(ids[o[:k]])
    # stack order: first-pushed-last; rebuild in left-to-right order
    return np.concatenate(out)


def _point_ubs(A, Btiles, nnear):
    """For each point in A [n,3]: an achievable nearest-distance^2 upper bound,
    the min over all points of the `nnear` nearest B-tiles (by center)."""
    bc = Btiles.mean(1)
    d = ((A[:, None, :] - bc[None, :, :]) ** 2).sum(-1)
    near = np.argpartition(d, nnear, axis=1)[:, :nnear]
    ub = np.full(len(A), np.inf)
    for j in range(nnear):
        sel = near[:, j]
        for g in np.unique(sel):
            m = sel == g
            dd = ((A[m][:, None, :] - Btiles[g][None, :, :]) ** 2).sum(-1).min(1)
            ub[m] = np.minimum(ub[m], dd)
    return ub


def _prep_batch(preds_b, gts_b):
    """Returns sorted perms, per-tile candidate chunk lists and split tables."""
    po = _kd_order(preds_b, GRP)
    go = _kd_order(gts_b, TIL)
    Ps = preds_b[po].astype(np.float64)
    Gs = gts_b[go].astype(np.float64)

    Pt = Ps.reshape(NGR, GRP, 3)
    Gt = Gs.reshape(NT, TIL, 3)
    plo, phi = Pt.min(1), Pt.max(1)
    glo, ghi = Gt.min(1), Gt.max(1)
    d1 = np.maximum(0.0, plo[None, :, :] - ghi[:, None, :])
    d2 = np.maximum(0.0, glo[:, None, :] - phi[None, :, :])
    LB = (np.maximum(d1, d2) ** 2).sum(-1)           # [NT, NGR]

    ub_t = _point_ubs(Gs, Pt, NNEAR)
    UB_T = ub_t.reshape(NT, TIL).max(1)              # [NT]
    ub_p = _point_ubs(Ps, Gt, NNEAR)
    UB_G = ub_p.reshape(NGR, GRP).max(1)             # [NGR]

    keep = LB <= np.maximum(UB_T[:, None], UB_G[None, :]) * (1.0 + 1e-6) + 1e-12

    # chunk lists: per tile, its fine groups packed into CHW-wide chunks
    chunks = []                                      # (tile, group_ids[GPC])
    for T in range(NT):
        gl = np.nonzero(keep[T])[0]
        padded = ((len(gl) + GPC - 1) // GPC) * GPC
        gl = np.resize(gl, padded)      # cycles values to pad
        for c in range(len(gl) // GPC):
            chunks.append((T, gl[c * GPC:(c + 1) * GPC]))

    # split tables over sorted points
    xsq = (Gs * Gs).sum(-1).astype(np.float32)
    ysq = (Ps * Ps).sum(-1).astype(np.float32)
    L = np.empty((5, N), np.float32)
    L[0:3] = (2.0 * Gs.T).astype(np.float32)
    L[3] = xsq
    L[4] = 1.0
    R = np.empty((5, N), np.float32)
    R[0:3] = Ps.T.astype(np.float32)
    R[3] = -1.0
    R[4] = -ysq
    sL, sR = _build_split_rows(L, R)                 # [24, N] bf16 each
    return dict(po=po, go=go, chunks=chunks, sL=sL, sR=sR)


def _prep(preds, gts):
    metas = [_prep_batch(preds[b], gts[b]) for b in range(B)]
    # distribute chunks to cores: core = 2*b + (tile >= NT//2)
    raw = []
    for b in range(B):
        for h in range(2):
            lo, hi = h * (NT // 2), (h + 1) * (NT // 2)
            raw.append([ch for ch in metas[b]['chunks'] if lo <= ch[0] < hi])
    nreal = [len(cc) for cc in raw]
    # 8 chunks per step (4 row-groups x 2 col-groups); 2 steps per in-DMA
    steps = max((n + 7) // 8 for n in nreal)
    steps += steps % 2
    in_maps = []
    core_data = []
    for c in range(NCORES):
        b = c // 2
        sL, sR = metas[b]['sL'], metas[b]['sR']
        cc = list(raw[c])
        while len(cc) < steps * 8:
            cc.append(cc[-1])
        # per step 1152 cols: slot (k, j): j-block partitions 32j+[0,24),
        # cols k*576 + [0:64 weights | 64:576 rhs]
        in0 = np.zeros((steps, 128, 1152), ml_dtypes.bfloat16)
        colmap = np.empty((steps * 8, CHW), np.int32)
        tileof = np.empty(steps * 8, np.int32)
        for i, (T, gl) in enumerate(cc):
            cols = (gl[:, None] * GRP + np.arange(GRP)[None, :]).ravel()
            s, slot = divmod(i, 8)
            k, j = divmod(slot, 4)
            o = k * 576
            in0[s, 32 * j:32 * j + KS, o:o + 64] = sL[:, T * TIL:(T + 1) * TIL]
            in0[s, 32 * j:32 * j + KS, o + 64:o + 576] = sR[:, cols]
            colmap[i] = cols
            tileof[i] = T
        in_maps.append({"in0": in0})
        core_data.append((cc, colmap, tileof))
    return metas, core_data, nreal, steps, in_maps


# ------------------------------------------------------------------- device
def _build_nc(steps):
    import concourse.mybir as mybir
    import concourse.tile as tile
    from concourse import bacc

    f32 = mybir.dt.float32
    bf16 = mybir.dt.bfloat16
    nc = bacc.Bacc("TRN2", target_bir_lowering=False, debug=False)

    # per step 8 chunks of [64, 512]: row-groups 32j (K=24) x col-groups 64k.
    # ACT casts psA (chunks j=0,1) -> fp8, DVE casts psB (j=2,3) -> fp8; the
    # separate PSUM tiles keep the two casts parallel. fp8 halves ship bytes;
    # rounding is monotone so the true row/col argmax still wins or ties.
    f8 = mybir.dt.float8e4
    in0_d = nc.dram_tensor("in0", [steps, 128, 1152], bf16, kind="ExternalInput")
    outa_d = nc.dram_tensor("outa", [steps // 2, 128, 2048], f8, kind="ExternalOutput")
    outb_d = nc.dram_tensor("outb", [steps // 2, 128, 2048], f8, kind="ExternalOutput")

    with tile.TileContext(nc) as tc, ExitStack() as ctx:
        io_pool = ctx.enter_context(tc.tile_pool(name="io", bufs=4))
        psum_pool = ctx.enter_context(tc.tile_pool(name="psum", bufs=2, space="PSUM"))
        stage_pool = ctx.enter_context(tc.tile_pool(name="stage", bufs=2))

        sta = stb = None
        for s in range(steps):
            t_in = io_pool.tile([128, 1152], bf16)
            nc.scalar.dma_start(t_in[:], in0_d[s, :, :])
            if s % 2 == 0:
                sta = stage_pool.tile([128, 2048], f8, tag="sta")
                stb = stage_pool.tile([128, 2048], f8, tag="stb")
            psa = psum_pool.tile([128, 1024], f32, tag="psa")
            psb = psum_pool.tile([128, 1024], f32, tag="psb")
            for k in range(2):
                for j in range(4):
                    o = k * 576
                    ps = psa if j < 2 else psb
                    nc.tensor.matmul(
                        ps[64 * k:64 * k + 64, (j % 2) * 512:(j % 2 + 1) * 512],
                        t_in[32 * j:32 * j + KS, o:o + 64],
                        t_in[32 * j:32 * j + KS, o + 64:o + 576],
                        start=True,
                        stop=True,
                        tile_position=(32 * j, 64 * k),
                    )
            ph = s % 2
            nc.scalar.copy(sta[:, ph * 1024:(ph + 1) * 1024], psa[:])
            nc.vector.tensor_copy(stb[:, ph * 1024:(ph + 1) * 1024], psb[:])
            if s % 2 == 1:
                nc.gpsimd.dma_start(outa_d[s // 2, :, :], sta[:])
                nc.sync.dma_start(outb_d[s // 2, :, :], stb[:])

    nc.compile()
    return nc


# ------------------------------------------------------------------ host: post
def _postprocess(preds, gts, normals, edges, results, metas, core_chunks, nreal):
    preds64 = preds.astype(np.float64)
    gts64 = gts.astype(np.float64)

    mins1 = np.empty((B, N), np.float64)
    mins2 = np.empty((B, N), np.float64)
    nearest_idx = np.empty((B, N), np.int64)

    for b in range(B):
        po, go = metas[b]['po'], metas[b]['go']
        # gather both cores' chunk values for this batch
        vals_all, cols_all, tile_all = [], [], []
        for h in range(2):
            c = 2 * b + h
            cc, colmap, tileof = core_chunks[c]
            # outa holds chunks j=0,1 of each step (k-halves in partitions);
            # outb j=2,3. chunk i = s*8 + k*4 + j.
            va = np.asarray(results[c]["outa"], ml_dtypes.float8_e4m3).astype(np.float32)
            vb = np.asarray(results[c]["outb"], ml_dtypes.float8_e4m3).astype(np.float32)
            # [s2, 2(k), 64, 2(ph), 2(j2), 512] -> [(s,k), j2, 64, 512]
            va = va.reshape(-1, 2, TIL, 2, 2, CHW).transpose(0, 3, 1, 4, 2, 5)
            vb = vb.reshape(-1, 2, TIL, 2, 2, CHW).transpose(0, 3, 1, 4, 2, 5)
            va = va.reshape(-1, 2, 2, TIL, CHW)          # [s, k, j2, 64, 512]
            vb = vb.reshape(-1, 2, 2, TIL, CHW)
            v = np.concatenate([va, vb], axis=2).reshape(-1, TIL, CHW)
            v = v[:nreal[c]]                             # [nch, 64, 512]
            vals_all.append(v)
            cols_all.append(colmap[:nreal[c]])
            tile_all.append(tileof[:nreal[c]])
        vals = np.concatenate(vals_all)                  # [M, 128, 512]
        cols = np.concatenate(cols_all)                  # [M, 512] sorted-p idx
        tils = np.concatenate(tile_all)                  # [M]

        # fp8 rounding is monotone: the true argmax always ties the quantized
        # max. Collect ALL tying candidates and resolve them exactly.
        G64, P64 = gts64[b], preds64[b]

        # ---- row path: per tile, max over its chunks' columns
        order = np.argsort(tils, kind='stable')
        vals_o, cols_o, tils_o = vals[order], cols[order], tils[order]
        bounds = np.searchsorted(tils_o, np.arange(NT + 1))
        for T in range(NT):
            i0, i1 = bounds[T], bounds[T + 1]
            v = vals_o[i0:i1]                            # [m, TIL, 512]
            flat = v.transpose(1, 0, 2).reshape(TIL, -1)
            mx = flat.max(1, keepdims=True)
            ti, pos = np.nonzero(flat == mx)             # tied candidates
            ci, cj = divmod(pos, CHW)
            srt_p = cols_o[i0:i1][ci, cj]
            t_orig = go[T * TIL + ti]
            p_orig = po[srt_p]
            d = ((G64[t_orig] - P64[p_orig]) ** 2).sum(-1)
            o3 = np.lexsort((d, ti))                     # per t: min d first
            tu, first = np.unique(ti[o3], return_index=True)
            sel = o3[first]
            rows = go[T * TIL + tu]
            mins2[b, rows] = d[sel]
            nearest_idx[b, rows] = p_orig[sel]

        # ---- col path: per sorted-p column, max over all (chunk, t)
        ncols = np.full(N, -np.inf, np.float32)
        np.maximum.at(ncols, cols.ravel(),
                      vals.max(1).ravel())               # fp8 col max
        cand_mask = vals == ncols[cols][:, None, :]      # [M, TIL, 512] ties
        mi, ti, cj = np.nonzero(cand_mask)
        srt_p = cols[mi, cj]
        srt_t = tils[mi] * TIL + ti
        d = ((G64[go[srt_t]] - P64[po[srt_p]]) ** 2).sum(-1)
        o2 = np.lexsort((d, srt_p))
        fc, first = np.unique(srt_p[o2], return_index=True)
        assert len(fc) == N, "column coverage hole"
        sel = o2[first]
        mins1[b, po[fc]] = d[sel]

    loss_1 = mins1.mean()
    loss_2 = mins2.mean()
    chamfer = loss_1 + loss_2

    e0 = edges[:, 0]
    e1 = edges[:, 1]
    edge_vectors = preds[:, e0, :] - preds[:, e1, :]
    edge_loss = (edge_vectors * edge_vectors).sum(axis=2).astype(np.float64).mean()

    normals_nearest = np.take_along_axis(normals, nearest_idx[:, :, None], axis=1)
    normals_edge = normals_nearest[:, e0, :]

    def l2n_dim1(v):
        n = np.sqrt((v * v).sum(axis=1, keepdims=True))
        return v / np.maximum(n, 1e-12)

    nn = l2n_dim1(normals_edge)
    nv = l2n_dim1(edge_vectors)
    cosines = np.abs((nn * nv).sum(axis=2))
    normal_cosine_loss = cosines.astype(np.float64).mean()

    return np.float32(
        30000.0 * chamfer + 240.0 * edge_loss + 200000.0 * normal_cosine_loss
    )


def kernel(preds, gts, normals, edges, _trace=False):
    from concourse.bass_utils import run_bass_kernel_spmd

    preds = np.asarray(preds, np.float32)
    gts = np.asarray(gts, np.float32)
    normals = np.asarray(normals, np.float32)
    edges = np.asarray(edges)

    metas, core_data, nreal, steps, in_maps = _prep(preds, gts)
    nc = _build_nc(steps)
    br = run_bass_kernel_spmd(nc, in_maps, list(range(NCORES)), trace=_trace)
    _LAST_RESULTS["bass_results"] = br
    return _postprocess(preds, gts, normals, edges, br.results,
                        metas, core_data, nreal)
